# revision 27
# baseline (speedup 1.0000x reference)
"""ArgmaxIOU kernel for 8 Trainium2 NeuronCores.

Data-parallel over batch: core i processes sample i (shapes hardcoded:
B=8, C=21, H=W=512). Raw Bass (explicit engines + semaphores).

Per 128x(21x512) pixel tile, per core:
  gpsimd: SWDGE DMA loads with inline f32->bf16 cast, each tile loaded as
          two class-halves (0:10 / 10:21) so the DVE can start its partial
          max-tree after half the transfer; 2 KB contiguous runs
  DVE:    pairwise-max tree over the 21 classes (bf16 2x mode), then
          is_equal against the broadcast max in four 128-column chunks ->
          one-hot argmax masks in the G-interleaved layout [TB, C, G]
  PE:     per chunk, matmul eqt^T @ eqp per G-block, accumulating a packed
          [84, 84] confusion matrix: conf[c,c'] = sum_g out[4c+g, 4c'+g]
  ACT:    final PSUM -> SBUF extraction (otherwise idle)
  sync:   result store (HWDGE)

The chunked eq/PE keeps the pipeline tail short: after the last load only
one 128-column chunk of work remains serialized behind another.

Host: gather the 8 packed matrices, fold G, compute mean IoU. bf16 argmax
quantization shifts the score by ~5e-4 relative.

Determinism notes (hardware-verified):
 - one DMA-completion semaphore per in-flight load (round-robin pool):
   summing all loads on one semaphore is racy across the 16 SDMA engines
 - contiguous runs stay >= 1 KB: shorter runs produced nondeterministic
   DMA completion behavior on this SWDGE cast path
"""

import sys

import numpy as np

for p in ("/opt/trn_rl_repo",):
    if p not in sys.path:
        sys.path.insert(0, p)

from contextlib import ExitStack

from concourse import bass, mybir
from concourse.bass_utils import run_bass_kernel_spmd

B = 8
C = 21
HW = 512 * 512
P = 128
Q = HW // P              # pixels per partition (2048)
G = 4                    # t-columns packed per matmul (4*21=84 <= 128)
M = G * C                # 84
T = 512                  # pixels per partition per load tile
J = Q // T               # 4 tiles
TB = T // G              # 128 matmul blocks per tile
CH = 4                   # eq/PE column chunks per tile
TBC = TB // CH           # 32 matmul blocks per chunk
TC = T // CH             # 128 columns per chunk
NSLOT = 2                # bf16 data tile slots (DMA runs ahead)
NEQ = 2                  # one-hot mask slots
NDM = 12                 # DMA-completion semaphore pool
INC_TILE = 2 * CH        # dve increments per tile

F32 = mybir.dt.float32
BF16 = mybir.dt.bfloat16


def build():
    nc = bass.Bass()
    pred = nc.declare_dram_parameter("prediction", [C, HW], F32, isOutput=False)
    targ = nc.declare_dram_parameter("target", [C, HW], F32, isOutput=False)
    out = nc.declare_dram_parameter("out", [M, M], F32, isOutput=True)

    # partition p owns pixels [p*Q, (p+1)*Q); tile j covers columns
    # [j*T, (j+1)*T) of every partition
    predv = pred[:].rearrange("c (p q) -> p c q", p=P)
    targv = targ[:].rearrange("c (p q) -> p c q", p=P)

    mx = mybir.AluOpType.max
    eqop = mybir.AluOpType.is_equal
    cp = mybir.ActivationFunctionType.Copy

    # load ledger: tile 0 is class-split (0:10 / 10:21) so the DVE can
    # start early; later tiles load whole (fewer DVE tree instructions).
    loads = []              # (j, tensor_id, c_lo, c_hi)
    for j in range(J):
        if j == 0:
            loads += [(j, 0, 0, 10), (j, 0, 10, 21),
                      (j, 1, 0, 10), (j, 1, 10, 21)]
        else:
            loads += [(j, 0, 0, 21), (j, 1, 0, 21)]
    lidx = {key: i for i, key in enumerate(loads)}

    def dm_of(key):
        i = lidx[key]
        return i % NDM, 16 * (i // NDM + 1)

    with ExitStack() as ctx:
        e_ = ctx.enter_context
        bt = e_(nc.sbuf_tensor("bt", [P, NSLOT, C, T], BF16))
        bp = e_(nc.sbuf_tensor("bp", [P, NSLOT, C, T], BF16))
        eqt = e_(nc.sbuf_tensor("eqt", [P, NEQ, TB, C, G], BF16))
        eqp = e_(nc.sbuf_tensor("eqp", [P, NEQ, TB, C, G], BF16))
        st = e_(nc.sbuf_tensor("st", [P, 10, T], BF16))
        sp = e_(nc.sbuf_tensor("sp", [P, 10, T], BF16))
        mt = e_(nc.sbuf_tensor("mt", [P, T], BF16))
        mp = e_(nc.sbuf_tensor("mp", [P, T], BF16))
        osb = e_(nc.sbuf_tensor("osb", [M, M], F32))
        conf = e_(nc.psum_tensor("conf", [M, M], F32))
        dms = [e_(nc.semaphore(f"dm{i}")) for i in range(NDM)]
        dve = e_(nc.semaphore("dve"))
        mm = e_(nc.semaphore("mm"))
        fin = e_(nc.semaphore("fin"))
        block = e_(nc.Block())

        srcs = {0: targv, 1: predv}
        dsts = {0: bt, 1: bp}

        @block.gpsimd
        def _(g):
            cur = -1
            for (j, tid, clo, chi) in loads:
                if j != cur:
                    cur = j
                    if j >= NSLOT:
                        g.wait_ge(dve, INC_TILE * (j - NSLOT + 1))
                s = j % NSLOT
                i, _ = dm_of((j, tid, clo, chi))
                g.dma_start(
                    out=dsts[tid][:, s, clo:chi, :],
                    in_=srcs[tid][:, clo:chi, j * T:(j + 1) * T],
                ).then_inc(dms[i], 16)

        @block.vector
        def _(v):
            def w(key):
                i, val = dm_of(key)
                v.wait_ge(dms[i], val)

            def treeA(data, scr):
                # partial max over classes 0:10 -> scr[:, 0:1, :]
                v.tensor_tensor(scr[:, 0:5, :], data[:, 0:5, :],
                                data[:, 5:10, :], mx)
                v.tensor_tensor(scr[:, 0:2, :], scr[:, 0:2, :],
                                scr[:, 2:4, :], mx)
                v.tensor_tensor(scr[:, 0:1, :], scr[:, 0:1, :],
                                scr[:, 1:2, :], mx)
                v.tensor_tensor(scr[:, 0:1, :], scr[:, 0:1, :],
                                scr[:, 4:5, :], mx)

            def treeB(data, scr, mo):
                # partial max over classes 10:21, fold with scr[:, 0:1, :]
                v.tensor_tensor(scr[:, 5:10, :], data[:, 10:15, :],
                                data[:, 15:20, :], mx)
                v.tensor_tensor(scr[:, 5:7, :], scr[:, 5:7, :],
                                scr[:, 7:9, :], mx)
                v.tensor_tensor(scr[:, 5:6, :], scr[:, 5:6, :],
                                scr[:, 6:7, :], mx)
                v.tensor_tensor(scr[:, 5:6, :], scr[:, 5:6, :],
                                scr[:, 9:10, :], mx)
                v.tensor_tensor(scr[:, 5:6, :], scr[:, 5:6, :],
                                data[:, 20:21, :], mx)
                v.tensor_tensor(mo[:].unsqueeze(1), scr[:, 0:1, :],
                                scr[:, 5:6, :], mx)

            def tree21(data, scr, mo):
                # max over all 21 class slices -> mo [P, T]
                v.tensor_tensor(scr[:, 0:10, :], data[:, 0:10, :],
                                data[:, 10:20, :], mx)
                v.tensor_tensor(scr[:, 0:5, :], scr[:, 0:5, :],
                                scr[:, 5:10, :], mx)
                v.tensor_tensor(scr[:, 0:2, :], scr[:, 0:2, :],
                                scr[:, 2:4, :], mx)
                v.tensor_tensor(scr[:, 0:1, :], scr[:, 0:1, :],
                                scr[:, 1:2, :], mx)
                v.tensor_tensor(scr[:, 0:1, :], scr[:, 0:1, :],
                                scr[:, 4:5, :], mx)
                v.tensor_tensor(mo[:].unsqueeze(1), scr[:, 0:1, :],
                                data[:, 20:21, :], mx)

            def eq_chunk(data, mo, dst, e, h):
                t0 = h * TC
                v.tensor_tensor(
                    dst[:, e, h * TBC:(h + 1) * TBC]
                        .rearrange("p tb c g -> p c tb g"),
                    data[:, :, t0:t0 + TC]
                        .rearrange("p c (tb g) -> p c tb g", g=G),
                    mo[:, t0:t0 + TC].rearrange("p (tb g) -> p tb g", g=G)
                        .unsqueeze(1).broadcast_to((P, C, TBC, G)),
                    eqop).then_inc(dve, 1)

            for j in range(J):
                s = j % NSLOT
                e = j % NEQ
                if j >= NEQ:
                    v.wait_ge(mm, j - NEQ + 1)           # PE done with eq slot
                if j == 0:
                    w((j, 0, 0, 10))
                    treeA(bt[:, s], st)
                    w((j, 0, 10, 21))
                    treeB(bt[:, s], st, mt)
                    w((j, 1, 0, 10))
                    treeA(bp[:, s], sp)
                    w((j, 1, 10, 21))
                    treeB(bp[:, s], sp, mp)
                else:
                    w((j, 0, 0, 21))
                    tree21(bt[:, s], st, mt)
                    w((j, 1, 0, 21))
                    tree21(bp[:, s], sp, mp)
                for h in range(CH):
                    eq_chunk(bt[:, s], mt, eqt, e, h)
                    eq_chunk(bp[:, s], mp, eqp, e, h)

        @block.scalar
        def _(sc):
            sc.wait_ge(mm, J)
            sc.activation(osb[:], conf[:], cp).then_inc(dve, 1)

        @block.sync
        def _(sy):
            sy.wait_ge(dve, INC_TILE * J + 1)            # osb written (ACT)
            sy.dma_start(out=out[:], in_=osb[:]).then_inc(fin, 16)
            sy.wait_ge(fin, 16)

        @block.tensor
        def _(te):
            for j in range(J):
                e = j % NEQ
                for h in range(CH):
                    te.wait_ge(dve, INC_TILE * j + 2 * (h + 1))
                    for tb in range(h * TBC, (h + 1) * TBC):
                        inst = te.matmul(
                            conf[:],
                            eqt[:, e, tb].rearrange("p c g -> p (c g)"),
                            eqp[:, e, tb].rearrange("p c g -> p (c g)"),
                            start=(j == 0 and tb == 0),
                            stop=(j == J - 1 and tb == TB - 1))
                        if tb == TB - 1:
                            inst.then_inc(mm, 1)

    return nc


def _score_from_packed(packed):
    """packed: [84, 84] f32 -> per-sample mean IoU (float64)."""
    x = packed.astype(np.float64).reshape(C, G, C, G)
    conf = np.einsum("igjg->ij", x)
    TP = np.diag(conf).copy()
    FN = conf.sum(axis=1) - TP
    FP = conf.sum(axis=0) - TP
    valid = TP > 0
    denom = TP + FN + FP
    iou = np.where(valid, TP / np.where(valid, denom, 1.0), 0.0)
    n_valid = max(float(valid.sum()), 1.0)
    return iou.sum() / n_valid


_NC_CACHE = {}


def _get_nc():
    if "nc" not in _NC_CACHE:
        _NC_CACHE["nc"] = build()
    return _NC_CACHE["nc"]


def run(prediction, target, trace=False):
    in_maps = []
    for i in range(B):
        in_maps.append({
            "prediction": np.ascontiguousarray(
                np.asarray(prediction[i], dtype=np.float32).reshape(C, HW)),
            "target": np.ascontiguousarray(
                np.asarray(target[i], dtype=np.float32).reshape(C, HW)),
        })
    res = run_bass_kernel_spmd(_get_nc(), in_maps, core_ids=list(range(B)),
                               trace=trace)
    scores = [_score_from_packed(res.results[i]["out"]) for i in range(B)]
    return np.float32(np.mean(scores)), res


def kernel(prediction, target):
    score, _ = run(prediction, target, trace=False)
    return score


# revision 29
# speedup vs baseline: 1.0117x; 1.0117x over previous
"""ArgmaxIOU kernel for 8 Trainium2 NeuronCores.

Data-parallel over batch: core i processes sample i (shapes hardcoded:
B=8, C=21, H=W=512). Raw Bass (explicit engines + semaphores).

Per 128x(21x512) pixel tile, per core:
  gpsimd: SWDGE DMA loads with inline f32->bf16 cast, each tile loaded as
          two class-halves (0:10 / 10:21) so the DVE can start its partial
          max-tree after half the transfer; 2 KB contiguous runs
  DVE:    pairwise-max tree over the 21 classes (bf16 2x mode), then
          is_equal against the broadcast max in four 128-column chunks ->
          one-hot argmax masks in the G-interleaved layout [TB, C, G]
  PE:     per chunk, matmul eqt^T @ eqp per G-block, accumulating a packed
          [84, 84] confusion matrix: conf[c,c'] = sum_g out[4c+g, 4c'+g]
  ACT:    final PSUM -> SBUF extraction (otherwise idle)
  sync:   result store (HWDGE)

The chunked eq/PE keeps the pipeline tail short: after the last load only
one 128-column chunk of work remains serialized behind another.

Host: gather the 8 packed matrices, fold G, compute mean IoU. bf16 argmax
quantization shifts the score by ~5e-4 relative.

Determinism notes (hardware-verified):
 - one DMA-completion semaphore per in-flight load (round-robin pool):
   summing all loads on one semaphore is racy across the 16 SDMA engines
 - contiguous runs stay >= 1 KB: shorter runs produced nondeterministic
   DMA completion behavior on this SWDGE cast path
"""

import sys

import numpy as np

for p in ("/opt/trn_rl_repo",):
    if p not in sys.path:
        sys.path.insert(0, p)

from contextlib import ExitStack

from concourse import bass, mybir
from concourse.bass_utils import run_bass_kernel_spmd

B = 8
C = 21
HW = 512 * 512
P = 128
Q = HW // P              # pixels per partition (2048)
G = 4                    # t-columns packed per matmul (4*21=84 <= 128)
M = G * C                # 84
T = 512                  # pixels per partition per load tile
J = Q // T               # 4 tiles
TB = T // G              # 128 matmul blocks per tile
CH = 4                   # eq/PE column chunks per tile
TBC = TB // CH           # 32 matmul blocks per chunk
TC = T // CH             # 128 columns per chunk
NSLOT = 2                # bf16 data tile slots (DMA runs ahead)
NEQ = 2                  # one-hot mask slots
NDM = 12                 # DMA-completion semaphore pool
INC_TILE = 2 * CH        # dve increments per tile

F32 = mybir.dt.float32
BF16 = mybir.dt.bfloat16


def build():
    nc = bass.Bass()
    pred = nc.declare_dram_parameter("prediction", [C, HW], F32, isOutput=False)
    targ = nc.declare_dram_parameter("target", [C, HW], F32, isOutput=False)
    out = nc.declare_dram_parameter("out", [M, M], F32, isOutput=True)

    # partition p owns pixels [p*Q, (p+1)*Q); tile j covers columns
    # [j*T, (j+1)*T) of every partition
    predv = pred[:].rearrange("c (p q) -> p c q", p=P)
    targv = targ[:].rearrange("c (p q) -> p c q", p=P)

    mx = mybir.AluOpType.max
    eqop = mybir.AluOpType.is_equal
    cp = mybir.ActivationFunctionType.Copy

    # load ledger: every tile/tensor is class-split (0:10 / 10:21) — the
    # DVE starts each partial max-tree after half the transfer, which
    # keeps the DMA/DVE pipeline tightly coupled.
    loads = []              # (j, tensor_id, c_lo, c_hi)
    for j in range(J):
        loads += [(j, 0, 0, 10), (j, 0, 10, 21),
                  (j, 1, 0, 10), (j, 1, 10, 21)]
    lidx = {key: i for i, key in enumerate(loads)}

    def dm_of(key):
        i = lidx[key]
        return i % NDM, 16 * (i // NDM + 1)

    with ExitStack() as ctx:
        e_ = ctx.enter_context
        bt = e_(nc.sbuf_tensor("bt", [P, NSLOT, C, T], BF16))
        bp = e_(nc.sbuf_tensor("bp", [P, NSLOT, C, T], BF16))
        eqt = e_(nc.sbuf_tensor("eqt", [P, NEQ, TB, C, G], BF16))
        eqp = e_(nc.sbuf_tensor("eqp", [P, NEQ, TB, C, G], BF16))
        st = e_(nc.sbuf_tensor("st", [P, 10, T], BF16))
        sp = e_(nc.sbuf_tensor("sp", [P, 10, T], BF16))
        mt = e_(nc.sbuf_tensor("mt", [P, T], BF16))
        mp = e_(nc.sbuf_tensor("mp", [P, T], BF16))
        osb = e_(nc.sbuf_tensor("osb", [M, M], F32))
        conf = e_(nc.psum_tensor("conf", [M, M], F32))
        dms = [e_(nc.semaphore(f"dm{i}")) for i in range(NDM)]
        dve = e_(nc.semaphore("dve"))
        mm = e_(nc.semaphore("mm"))
        fin = e_(nc.semaphore("fin"))
        block = e_(nc.Block())

        srcs = {0: targv, 1: predv}
        dsts = {0: bt, 1: bp}

        @block.gpsimd
        def _(g):
            cur = -1
            for (j, tid, clo, chi) in loads:
                if j != cur:
                    cur = j
                    if j >= NSLOT:
                        g.wait_ge(dve, INC_TILE * (j - NSLOT + 1))
                s = j % NSLOT
                i, _ = dm_of((j, tid, clo, chi))
                g.dma_start(
                    out=dsts[tid][:, s, clo:chi, :],
                    in_=srcs[tid][:, clo:chi, j * T:(j + 1) * T],
                ).then_inc(dms[i], 16)

        @block.vector
        def _(v):
            def w(key):
                i, val = dm_of(key)
                v.wait_ge(dms[i], val)

            def treeA(data, scr):
                # partial max over classes 0:10 -> scr[:, 0:1, :]
                v.tensor_tensor(scr[:, 0:5, :], data[:, 0:5, :],
                                data[:, 5:10, :], mx)
                v.tensor_tensor(scr[:, 0:2, :], scr[:, 0:2, :],
                                scr[:, 2:4, :], mx)
                v.tensor_tensor(scr[:, 0:1, :], scr[:, 0:1, :],
                                scr[:, 1:2, :], mx)
                v.tensor_tensor(scr[:, 0:1, :], scr[:, 0:1, :],
                                scr[:, 4:5, :], mx)

            def treeB(data, scr, mo):
                # partial max over classes 10:21, fold with scr[:, 0:1, :]
                v.tensor_tensor(scr[:, 5:10, :], data[:, 10:15, :],
                                data[:, 15:20, :], mx)
                v.tensor_tensor(scr[:, 5:7, :], scr[:, 5:7, :],
                                scr[:, 7:9, :], mx)
                v.tensor_tensor(scr[:, 5:6, :], scr[:, 5:6, :],
                                scr[:, 6:7, :], mx)
                v.tensor_tensor(scr[:, 5:6, :], scr[:, 5:6, :],
                                scr[:, 9:10, :], mx)
                v.tensor_tensor(scr[:, 5:6, :], scr[:, 5:6, :],
                                data[:, 20:21, :], mx)
                v.tensor_tensor(mo[:].unsqueeze(1), scr[:, 0:1, :],
                                scr[:, 5:6, :], mx)

            def tree21(data, scr, mo):
                # max over all 21 class slices -> mo [P, T]
                v.tensor_tensor(scr[:, 0:10, :], data[:, 0:10, :],
                                data[:, 10:20, :], mx)
                v.tensor_tensor(scr[:, 0:5, :], scr[:, 0:5, :],
                                scr[:, 5:10, :], mx)
                v.tensor_tensor(scr[:, 0:2, :], scr[:, 0:2, :],
                                scr[:, 2:4, :], mx)
                v.tensor_tensor(scr[:, 0:1, :], scr[:, 0:1, :],
                                scr[:, 1:2, :], mx)
                v.tensor_tensor(scr[:, 0:1, :], scr[:, 0:1, :],
                                scr[:, 4:5, :], mx)
                v.tensor_tensor(mo[:].unsqueeze(1), scr[:, 0:1, :],
                                data[:, 20:21, :], mx)

            def eq_chunk(data, mo, dst, e, h):
                t0 = h * TC
                v.tensor_tensor(
                    dst[:, e, h * TBC:(h + 1) * TBC]
                        .rearrange("p tb c g -> p c tb g"),
                    data[:, :, t0:t0 + TC]
                        .rearrange("p c (tb g) -> p c tb g", g=G),
                    mo[:, t0:t0 + TC].rearrange("p (tb g) -> p tb g", g=G)
                        .unsqueeze(1).broadcast_to((P, C, TBC, G)),
                    eqop).then_inc(dve, 1)

            for j in range(J):
                s = j % NSLOT
                e = j % NEQ
                if j >= NEQ:
                    v.wait_ge(mm, j - NEQ + 1)           # PE done with eq slot
                w((j, 0, 0, 10))
                treeA(bt[:, s], st)
                w((j, 0, 10, 21))
                treeB(bt[:, s], st, mt)
                w((j, 1, 0, 10))
                treeA(bp[:, s], sp)
                w((j, 1, 10, 21))
                treeB(bp[:, s], sp, mp)
                for h in range(CH):
                    eq_chunk(bt[:, s], mt, eqt, e, h)
                    eq_chunk(bp[:, s], mp, eqp, e, h)

        @block.scalar
        def _(sc):
            sc.wait_ge(mm, J)
            sc.activation(osb[:], conf[:], cp).then_inc(dve, 1)

        @block.sync
        def _(sy):
            sy.wait_ge(dve, INC_TILE * J + 1)            # osb written (ACT)
            sy.dma_start(out=out[:], in_=osb[:]).then_inc(fin, 16)
            sy.wait_ge(fin, 16)

        @block.tensor
        def _(te):
            for j in range(J):
                e = j % NEQ
                for h in range(CH):
                    te.wait_ge(dve, INC_TILE * j + 2 * (h + 1))
                    for tb in range(h * TBC, (h + 1) * TBC):
                        inst = te.matmul(
                            conf[:],
                            eqt[:, e, tb].rearrange("p c g -> p (c g)"),
                            eqp[:, e, tb].rearrange("p c g -> p (c g)"),
                            start=(j == 0 and tb == 0),
                            stop=(j == J - 1 and tb == TB - 1))
                        if tb == TB - 1:
                            inst.then_inc(mm, 1)

    return nc


def _score_from_packed(packed):
    """packed: [84, 84] f32 -> per-sample mean IoU (float64)."""
    x = packed.astype(np.float64).reshape(C, G, C, G)
    conf = np.einsum("igjg->ij", x)
    TP = np.diag(conf).copy()
    FN = conf.sum(axis=1) - TP
    FP = conf.sum(axis=0) - TP
    valid = TP > 0
    denom = TP + FN + FP
    iou = np.where(valid, TP / np.where(valid, denom, 1.0), 0.0)
    n_valid = max(float(valid.sum()), 1.0)
    return iou.sum() / n_valid


_NC_CACHE = {}


def _get_nc():
    if "nc" not in _NC_CACHE:
        _NC_CACHE["nc"] = build()
    return _NC_CACHE["nc"]


def run(prediction, target, trace=False):
    in_maps = []
    for i in range(B):
        in_maps.append({
            "prediction": np.ascontiguousarray(
                np.asarray(prediction[i], dtype=np.float32).reshape(C, HW)),
            "target": np.ascontiguousarray(
                np.asarray(target[i], dtype=np.float32).reshape(C, HW)),
        })
    res = run_bass_kernel_spmd(_get_nc(), in_maps, core_ids=list(range(B)),
                               trace=trace)
    scores = [_score_from_packed(res.results[i]["out"]) for i in range(B)]
    return np.float32(np.mean(scores)), res


def kernel(prediction, target):
    score, _ = run(prediction, target, trace=False)
    return score


# revision 30
# speedup vs baseline: 1.1710x; 1.1574x over previous
"""ArgmaxIOU kernel for 8 Trainium2 NeuronCores.

Data-parallel over batch: core i processes sample i (shapes hardcoded:
B=8, C=21, H=W=512). Raw Bass (explicit engines + semaphores).

Per 128x(21x256) pixel tile, per core:
  gpsimd: SWDGE DMA loads with inline f32->bf16 cast (halves SBUF traffic,
          no separate convert stage)
  DVE:    pairwise-max tree over the 21 classes (bf16 2x mode), then
          is_equal against the broadcast max -> one-hot argmax mask,
          scattered into the G-interleaved matmul layout [TB, C, G]
  PE:     matmul eqt^T @ eqp per G-block, accumulating a packed [84, 84]
          confusion matrix in PSUM: conf[c,c'] = sum_g out[4c+g, 4c'+g]
  ACT:    final PSUM -> SBUF extraction (otherwise idle)

Tile 0's loads are split by class range (10 + 11 classes, full 128
partitions, >=1 KB runs) so the first max-tree starts after roughly half
the first transfer — shortens pipeline fill by ~10 us.

Host: gather the 8 packed matrices, fold G, compute mean IoU. bf16 argmax
quantization shifts the score by ~5e-4 relative (bf16 ties are rare and
wash out of the large confusion counts).

Determinism notes (hardware-verified):
 - one DMA-completion semaphore per in-flight load (round-robin pool):
   summing all loads on one semaphore is racy across the 16 SDMA engines
 - uniform 256-column tiles only: sub-256-column tiles produced
   nondeterministic DMA completion behavior on this SWDGE cast path
"""

import sys

import numpy as np

for p in ("/opt/trn_rl_repo",):
    if p not in sys.path:
        sys.path.insert(0, p)

from contextlib import ExitStack

from concourse import bass, mybir
from concourse.bass_utils import run_bass_kernel_spmd

B = 8
C = 21
HW = 512 * 512
P = 128
Q = HW // P              # pixels per partition (2048)
G = 4                    # t-columns packed per matmul (4*21=84 <= 128)
M = G * C                # 84
T = 256                  # pixels per partition per tile
J = Q // T               # 8 tiles
TB = T // G              # 64 matmul blocks per tile
NSLOT = 4                # bf16 data tile slots (DMA runs ahead)
NEQ = 3                  # one-hot mask slots
NDM = 12                 # DMA-completion semaphore pool

F32 = mybir.dt.float32
BF16 = mybir.dt.bfloat16


def build():
    nc = bass.Bass()
    pred = nc.declare_dram_parameter("prediction", [C, HW], F32, isOutput=False)
    targ = nc.declare_dram_parameter("target", [C, HW], F32, isOutput=False)
    out = nc.declare_dram_parameter("out", [M, M], F32, isOutput=True)

    # partition p owns pixels [p*Q, (p+1)*Q); tile j covers columns
    # [j*T, (j+1)*T) of every partition
    predv = pred[:].rearrange("c (p q) -> p c q", p=P)
    targv = targ[:].rearrange("c (p q) -> p c q", p=P)

    mx = mybir.AluOpType.max
    eqop = mybir.AluOpType.is_equal
    cp = mybir.ActivationFunctionType.Copy

    # load ledger: tile 0 is class-split (0:10 / 10:21) per tensor; the
    # rest load all 21 classes at once. Issue order == list order.
    loads = []              # (j, tensor_id, c_lo, c_hi)
    for j in range(J):
        if j == 0:
            loads += [(j, 0, 0, 10), (j, 0, 10, 21),
                      (j, 1, 0, 10), (j, 1, 10, 21)]
        else:
            loads += [(j, 0, 0, 21), (j, 1, 0, 21)]
    lidx = {key: i for i, key in enumerate(loads)}

    def dm_of(key):
        i = lidx[key]
        return i, 16 * (i // NDM + 1)

    with ExitStack() as ctx:
        e_ = ctx.enter_context
        bt = e_(nc.sbuf_tensor("bt", [P, NSLOT, C, T], BF16))
        bp = e_(nc.sbuf_tensor("bp", [P, NSLOT, C, T], BF16))
        eqt = e_(nc.sbuf_tensor("eqt", [P, NEQ, TB, C, G], BF16))
        eqp = e_(nc.sbuf_tensor("eqp", [P, NEQ, TB, C, G], BF16))
        st = e_(nc.sbuf_tensor("st", [P, 10, T], BF16))
        sp = e_(nc.sbuf_tensor("sp", [P, 10, T], BF16))
        mt = e_(nc.sbuf_tensor("mt", [P, T], BF16))
        mp = e_(nc.sbuf_tensor("mp", [P, T], BF16))
        osb = e_(nc.sbuf_tensor("osb", [M, M], F32))
        conf = e_(nc.psum_tensor("conf", [M, M], F32))
        dms = [e_(nc.semaphore(f"dm{i}")) for i in range(NDM)]
        dve = e_(nc.semaphore("dve"))
        mm = e_(nc.semaphore("mm"))
        fin = e_(nc.semaphore("fin"))
        block = e_(nc.Block())

        srcs = {0: targv, 1: predv}
        dsts = {0: bt, 1: bp}

        @block.gpsimd
        def _(g):
            cur = -1
            for (j, tid, clo, chi) in loads:
                if j != cur:
                    cur = j
                    if j >= NSLOT:
                        g.wait_ge(dve, 4 * (j - NSLOT + 1))
                s = j % NSLOT
                i, _ = dm_of((j, tid, clo, chi))
                g.dma_start(
                    out=dsts[tid][:, s, clo:chi, :],
                    in_=srcs[tid][:, clo:chi, j * T:(j + 1) * T],
                ).then_inc(dms[i % NDM], 16)
            g.wait_ge(dve, 4 * J + 1)                    # osb written (ACT)
            g.dma_start(out=out[:], in_=osb[:]).then_inc(fin, 16)
            g.wait_ge(fin, 16)

        @block.vector
        def _(v):
            def w(key):
                i, val = dm_of(key)
                v.wait_ge(dms[i % NDM], val)

            def tree21(data, scr, mo):
                # max over all 21 class slices -> mo [P, T]
                v.tensor_tensor(scr[:, 0:10, :], data[:, 0:10, :],
                                data[:, 10:20, :], mx)
                v.tensor_tensor(scr[:, 0:5, :], scr[:, 0:5, :],
                                scr[:, 5:10, :], mx)
                v.tensor_tensor(scr[:, 0:2, :], scr[:, 0:2, :],
                                scr[:, 2:4, :], mx)
                v.tensor_tensor(scr[:, 0:1, :], scr[:, 0:1, :],
                                scr[:, 1:2, :], mx)
                v.tensor_tensor(scr[:, 0:1, :], scr[:, 0:1, :],
                                scr[:, 4:5, :], mx)
                v.tensor_tensor(mo[:].unsqueeze(1), scr[:, 0:1, :],
                                data[:, 20:21, :], mx)

            def treeA(data, scr):
                # partial max over classes 0:10 -> scr[:, 0:1, :]
                v.tensor_tensor(scr[:, 0:5, :], data[:, 0:5, :],
                                data[:, 5:10, :], mx)
                v.tensor_tensor(scr[:, 0:2, :], scr[:, 0:2, :],
                                scr[:, 2:4, :], mx)
                v.tensor_tensor(scr[:, 0:1, :], scr[:, 0:1, :],
                                scr[:, 1:2, :], mx)
                v.tensor_tensor(scr[:, 0:1, :], scr[:, 0:1, :],
                                scr[:, 4:5, :], mx)

            def treeB(data, scr, mo):
                # partial max over classes 10:21 -> fold with scr[:, 0:1, :]
                v.tensor_tensor(scr[:, 5:10, :], data[:, 10:15, :],
                                data[:, 15:20, :], mx)
                v.tensor_tensor(scr[:, 5:7, :], scr[:, 5:7, :],
                                scr[:, 7:9, :], mx)
                v.tensor_tensor(scr[:, 5:6, :], scr[:, 5:6, :],
                                scr[:, 6:7, :], mx)
                v.tensor_tensor(scr[:, 5:6, :], scr[:, 5:6, :],
                                scr[:, 9:10, :], mx)
                v.tensor_tensor(scr[:, 5:6, :], scr[:, 5:6, :],
                                data[:, 20:21, :], mx)
                v.tensor_tensor(mo[:].unsqueeze(1), scr[:, 0:1, :],
                                scr[:, 5:6, :], mx)

            HTB = TB // 2
            HT = T // 2

            def eq(data, mo, dst, e):
                for h in range(2):
                    t0 = h * HT
                    v.tensor_tensor(
                        dst[:, e, h * HTB:(h + 1) * HTB]
                            .rearrange("p tb c g -> p c tb g"),
                        data[:, :, t0:t0 + HT]
                            .rearrange("p c (tb g) -> p c tb g", g=G),
                        mo[:, t0:t0 + HT].rearrange("p (tb g) -> p tb g", g=G)
                            .unsqueeze(1).broadcast_to((P, C, HTB, 4)),
                        eqop).then_inc(dve, 1)

            for j in range(J):
                s = j % NSLOT
                e = j % NEQ
                if j >= NEQ:
                    v.wait_ge(mm, j - NEQ + 1)           # PE done with eq slot
                if j == 0:
                    w((j, 0, 0, 10))
                    treeA(bt[:, s], st)
                    w((j, 0, 10, 21))
                    treeB(bt[:, s], st, mt)
                    eq(bt[:, s], mt, eqt, e)
                    w((j, 1, 0, 10))
                    treeA(bp[:, s], sp)
                    w((j, 1, 10, 21))
                    treeB(bp[:, s], sp, mp)
                    eq(bp[:, s], mp, eqp, e)
                else:
                    w((j, 0, 0, 21))
                    tree21(bt[:, s], st, mt)
                    eq(bt[:, s], mt, eqt, e)
                    w((j, 1, 0, 21))
                    tree21(bp[:, s], sp, mp)
                    eq(bp[:, s], mp, eqp, e)

        @block.scalar
        def _(sc):
            sc.wait_ge(mm, J)
            sc.activation(osb[:], conf[:], cp).then_inc(dve, 1)

        @block.tensor
        def _(te):
            for j in range(J):
                e = j % NEQ
                for h in range(2):
                    te.wait_ge(dve, 4 * j + 3 + h)
                    for tb in range(h * (TB // 2), (h + 1) * (TB // 2)):
                        inst = te.matmul(
                            conf[:],
                            eqt[:, e, tb].rearrange("p c g -> p (c g)"),
                            eqp[:, e, tb].rearrange("p c g -> p (c g)"),
                            start=(j == 0 and tb == 0),
                            stop=(j == J - 1 and tb == TB - 1))
                        if tb == TB - 1:
                            inst.then_inc(mm, 1)

    return nc


def _score_from_packed(packed):
    """packed: [84, 84] f32 -> per-sample mean IoU (float64)."""
    x = packed.astype(np.float64).reshape(C, G, C, G)
    conf = np.einsum("igjg->ij", x)
    TP = np.diag(conf).copy()
    FN = conf.sum(axis=1) - TP
    FP = conf.sum(axis=0) - TP
    valid = TP > 0
    denom = TP + FN + FP
    iou = np.where(valid, TP / np.where(valid, denom, 1.0), 0.0)
    n_valid = max(float(valid.sum()), 1.0)
    return iou.sum() / n_valid


_NC_CACHE = {}


def _get_nc():
    if "nc" not in _NC_CACHE:
        _NC_CACHE["nc"] = build()
    return _NC_CACHE["nc"]


def run(prediction, target, trace=False):
    in_maps = []
    for i in range(B):
        in_maps.append({
            "prediction": np.ascontiguousarray(
                np.asarray(prediction[i], dtype=np.float32).reshape(C, HW)),
            "target": np.ascontiguousarray(
                np.asarray(target[i], dtype=np.float32).reshape(C, HW)),
        })
    res = run_bass_kernel_spmd(_get_nc(), in_maps, core_ids=list(range(B)),
                               trace=trace)
    scores = [_score_from_packed(res.results[i]["out"]) for i in range(B)]
    return np.float32(np.mean(scores)), res


def kernel(prediction, target):
    score, _ = run(prediction, target, trace=False)
    return score


# revision 33
# speedup vs baseline: 1.2060x; 1.0298x over previous
"""ArgmaxIOU kernel for 8 Trainium2 NeuronCores.

Data-parallel over batch: core i processes sample i (shapes hardcoded:
B=8, C=21, H=W=512). Raw Bass (explicit engines + semaphores).

Per 128x(21x256) pixel tile, per core:
  gpsimd: SWDGE DMA loads with inline f32->bf16 cast (halves SBUF traffic,
          no separate convert stage)
  DVE:    pairwise-max tree over the 21 classes (bf16 2x mode), then
          is_equal against the broadcast max -> one-hot argmax mask,
          scattered into the G-interleaved matmul layout [TB, C, G]
  PE:     matmul eqt^T @ eqp per G-block (consumed in half-tile chunks to
          keep the pipeline tail short), accumulating a packed [84, 84]
          confusion matrix in PSUM: conf[c,c'] = sum_g out[4c+g, 4c'+g]
  ACT:    final PSUM -> SBUF extraction (otherwise idle)

Tile 0's loads are split by class range (10 + 11 classes, full 128
partitions, >=1 KB runs) so the first max-tree starts after roughly half
the first transfer — shortens pipeline fill by ~10 us. Measured ~142 us
on hardware vs a ~123 us HBM roofline (44 MB/core at ~358 GB/s).

Host: gather the 8 packed matrices, fold G, compute mean IoU. bf16 argmax
quantization shifts the score by ~5e-4 relative (bf16 ties are rare and
wash out of the large confusion counts).

Determinism notes (hardware-verified):
 - one DMA-completion semaphore per in-flight load (round-robin pool):
   summing all loads on one semaphore is racy across the 16 SDMA engines
 - uniform 256-column tiles only: sub-256-column tiles produced
   nondeterministic DMA completion behavior on this SWDGE cast path
"""

import sys

import numpy as np

for p in ("/opt/trn_rl_repo",):
    if p not in sys.path:
        sys.path.insert(0, p)

from contextlib import ExitStack

from concourse import bass, mybir
from concourse.bass_utils import run_bass_kernel_spmd

B = 8
C = 21
HW = 512 * 512
P = 128
Q = HW // P              # pixels per partition (2048)
G = 4                    # t-columns packed per matmul (4*21=84 <= 128)
M = G * C                # 84
T = 256                  # pixels per partition per tile
J = Q // T               # 8 tiles
TB = T // G              # 64 matmul blocks per tile
NSLOT = 6                # bf16 data tile slots (DMA runs ahead)
NEQ = 3                  # one-hot mask slots
NDM = 12                 # DMA-completion semaphore pool

F32 = mybir.dt.float32
BF16 = mybir.dt.bfloat16


def build():
    nc = bass.Bass()
    pred = nc.declare_dram_parameter("prediction", [C, HW], F32, isOutput=False)
    targ = nc.declare_dram_parameter("target", [C, HW], F32, isOutput=False)
    out = nc.declare_dram_parameter("out", [M, M], F32, isOutput=True)

    # partition p owns pixels [p*Q, (p+1)*Q); tile j covers columns
    # [j*T, (j+1)*T) of every partition
    predv = pred[:].rearrange("c (p q) -> p c q", p=P)
    targv = targ[:].rearrange("c (p q) -> p c q", p=P)

    mx = mybir.AluOpType.max
    eqop = mybir.AluOpType.is_equal
    cp = mybir.ActivationFunctionType.Copy

    # load ledger: tile 0 is class-split (0:10 / 10:21) per tensor; the
    # rest load all 21 classes at once. Issue order == list order.
    loads = []              # (j, tensor_id, c_lo, c_hi)
    for j in range(J):
        if j == 0:
            loads += [(j, 0, 0, 10), (j, 0, 10, 21),
                      (j, 1, 0, 10), (j, 1, 10, 21)]
        else:
            loads += [(j, 0, 0, 21), (j, 1, 0, 21)]
    lidx = {key: i for i, key in enumerate(loads)}

    def dm_of(key):
        i = lidx[key]
        return i, 16 * (i // NDM + 1)

    with ExitStack() as ctx:
        e_ = ctx.enter_context
        bt = e_(nc.sbuf_tensor("bt", [P, NSLOT, C, T], BF16))
        bp = e_(nc.sbuf_tensor("bp", [P, NSLOT, C, T], BF16))
        eqt = e_(nc.sbuf_tensor("eqt", [P, NEQ, TB, C, G], BF16))
        eqp = e_(nc.sbuf_tensor("eqp", [P, NEQ, TB, C, G], BF16))
        st = e_(nc.sbuf_tensor("st", [P, 10, T], BF16))
        sp = e_(nc.sbuf_tensor("sp", [P, 10, T], BF16))
        mt = e_(nc.sbuf_tensor("mt", [P, T], BF16))
        mp = e_(nc.sbuf_tensor("mp", [P, T], BF16))
        osb = e_(nc.sbuf_tensor("osb", [M, M], F32))
        conf = e_(nc.psum_tensor("conf", [M, M], F32))
        dms = [e_(nc.semaphore(f"dm{i}")) for i in range(NDM)]
        dve = e_(nc.semaphore("dve"))
        mm = e_(nc.semaphore("mm"))
        fin = e_(nc.semaphore("fin"))
        block = e_(nc.Block())

        srcs = {0: targv, 1: predv}
        dsts = {0: bt, 1: bp}

        @block.gpsimd
        def _(g):
            cur = -1
            for (j, tid, clo, chi) in loads:
                if j != cur:
                    cur = j
                    if j >= NSLOT:
                        g.wait_ge(dve, 4 * (j - NSLOT + 1))
                s = j % NSLOT
                i, _ = dm_of((j, tid, clo, chi))
                g.dma_start(
                    out=dsts[tid][:, s, clo:chi, :],
                    in_=srcs[tid][:, clo:chi, j * T:(j + 1) * T],
                ).then_inc(dms[i % NDM], 16)
            g.wait_ge(dve, 4 * J + 1)                    # osb written (ACT)
            g.dma_start(out=out[:], in_=osb[:]).then_inc(fin, 16)
            g.wait_ge(fin, 16)

        @block.vector
        def _(v):
            def w(key):
                i, val = dm_of(key)
                v.wait_ge(dms[i % NDM], val)

            def tree21(data, scr, mo):
                # max over all 21 class slices -> mo [P, T]
                v.tensor_tensor(scr[:, 0:10, :], data[:, 0:10, :],
                                data[:, 10:20, :], mx)
                v.tensor_tensor(scr[:, 0:5, :], scr[:, 0:5, :],
                                scr[:, 5:10, :], mx)
                v.tensor_tensor(scr[:, 0:2, :], scr[:, 0:2, :],
                                scr[:, 2:4, :], mx)
                v.tensor_tensor(scr[:, 0:1, :], scr[:, 0:1, :],
                                scr[:, 1:2, :], mx)
                v.tensor_tensor(scr[:, 0:1, :], scr[:, 0:1, :],
                                scr[:, 4:5, :], mx)
                v.tensor_tensor(mo[:].unsqueeze(1), scr[:, 0:1, :],
                                data[:, 20:21, :], mx)

            def treeA(data, scr):
                # partial max over classes 0:10 -> scr[:, 0:1, :]
                v.tensor_tensor(scr[:, 0:5, :], data[:, 0:5, :],
                                data[:, 5:10, :], mx)
                v.tensor_tensor(scr[:, 0:2, :], scr[:, 0:2, :],
                                scr[:, 2:4, :], mx)
                v.tensor_tensor(scr[:, 0:1, :], scr[:, 0:1, :],
                                scr[:, 1:2, :], mx)
                v.tensor_tensor(scr[:, 0:1, :], scr[:, 0:1, :],
                                scr[:, 4:5, :], mx)

            def treeB(data, scr, mo):
                # partial max over classes 10:21 -> fold with scr[:, 0:1, :]
                v.tensor_tensor(scr[:, 5:10, :], data[:, 10:15, :],
                                data[:, 15:20, :], mx)
                v.tensor_tensor(scr[:, 5:7, :], scr[:, 5:7, :],
                                scr[:, 7:9, :], mx)
                v.tensor_tensor(scr[:, 5:6, :], scr[:, 5:6, :],
                                scr[:, 6:7, :], mx)
                v.tensor_tensor(scr[:, 5:6, :], scr[:, 5:6, :],
                                scr[:, 9:10, :], mx)
                v.tensor_tensor(scr[:, 5:6, :], scr[:, 5:6, :],
                                data[:, 20:21, :], mx)
                v.tensor_tensor(mo[:].unsqueeze(1), scr[:, 0:1, :],
                                scr[:, 5:6, :], mx)

            HTB = TB // 2
            HT = T // 2

            def eq(data, mo, dst, e):
                for h in range(2):
                    t0 = h * HT
                    v.tensor_tensor(
                        dst[:, e, h * HTB:(h + 1) * HTB]
                            .rearrange("p tb c g -> p c tb g"),
                        data[:, :, t0:t0 + HT]
                            .rearrange("p c (tb g) -> p c tb g", g=G),
                        mo[:, t0:t0 + HT].rearrange("p (tb g) -> p tb g", g=G)
                            .unsqueeze(1).broadcast_to((P, C, HTB, 4)),
                        eqop).then_inc(dve, 1)

            for j in range(J):
                s = j % NSLOT
                e = j % NEQ
                if j >= NEQ:
                    v.wait_ge(mm, j - NEQ + 1)           # PE done with eq slot
                if j == 0:
                    w((j, 0, 0, 10))
                    treeA(bt[:, s], st)
                    w((j, 0, 10, 21))
                    treeB(bt[:, s], st, mt)
                    eq(bt[:, s], mt, eqt, e)
                    w((j, 1, 0, 10))
                    treeA(bp[:, s], sp)
                    w((j, 1, 10, 21))
                    treeB(bp[:, s], sp, mp)
                    eq(bp[:, s], mp, eqp, e)
                else:
                    w((j, 0, 0, 21))
                    tree21(bt[:, s], st, mt)
                    eq(bt[:, s], mt, eqt, e)
                    w((j, 1, 0, 21))
                    tree21(bp[:, s], sp, mp)
                    eq(bp[:, s], mp, eqp, e)

        @block.scalar
        def _(sc):
            sc.wait_ge(mm, J)
            sc.activation(osb[:], conf[:], cp).then_inc(dve, 1)

        @block.tensor
        def _(te):
            for j in range(J):
                e = j % NEQ
                for h in range(2):
                    te.wait_ge(dve, 4 * j + 3 + h)
                    for tb in range(h * (TB // 2), (h + 1) * (TB // 2)):
                        inst = te.matmul(
                            conf[:],
                            eqt[:, e, tb].rearrange("p c g -> p (c g)"),
                            eqp[:, e, tb].rearrange("p c g -> p (c g)"),
                            start=(j == 0 and tb == 0),
                            stop=(j == J - 1 and tb == TB - 1))
                        if tb == TB - 1:
                            inst.then_inc(mm, 1)

    return nc


def _score_from_packed(packed):
    """packed: [84, 84] f32 -> per-sample mean IoU (float64)."""
    x = packed.astype(np.float64).reshape(C, G, C, G)
    conf = np.einsum("igjg->ij", x)
    TP = np.diag(conf).copy()
    FN = conf.sum(axis=1) - TP
    FP = conf.sum(axis=0) - TP
    valid = TP > 0
    denom = TP + FN + FP
    iou = np.where(valid, TP / np.where(valid, denom, 1.0), 0.0)
    n_valid = max(float(valid.sum()), 1.0)
    return iou.sum() / n_valid


_NC_CACHE = {}


def _get_nc():
    if "nc" not in _NC_CACHE:
        _NC_CACHE["nc"] = build()
    return _NC_CACHE["nc"]


def run(prediction, target, trace=False):
    in_maps = []
    for i in range(B):
        in_maps.append({
            "prediction": np.ascontiguousarray(
                np.asarray(prediction[i], dtype=np.float32).reshape(C, HW)),
            "target": np.ascontiguousarray(
                np.asarray(target[i], dtype=np.float32).reshape(C, HW)),
        })
    res = run_bass_kernel_spmd(_get_nc(), in_maps, core_ids=list(range(B)),
                               trace=trace)
    scores = [_score_from_packed(res.results[i]["out"]) for i in range(B)]
    return np.float32(np.mean(scores)), res


def kernel(prediction, target):
    score, _ = run(prediction, target, trace=False)
    return score
